# revision 32
# baseline (speedup 1.0000x reference)
"""Trainium2 Bass kernel for BioBERT-ARG-GNN (gated pooling + 2-layer GCN + MLP head).

Strategy (v4): pure data parallel over batch B=64 across 8 NeuronCores (8
graphs per core).  Host precomputes index-derived structures (one-hot
pooling matrix P' with 1/cnt and D^-1/2 folded in, normalized adjacency
\hat A = D^-1/2 (A+I) D^-1/2) and ships them bf16 together with a bf16
TRANSPOSED copy of last_hidden (lhT, [hidden, tokens]) in ONE mega-tensor
per graph.  The transposed layout lets BOTH the gate logits (wr . lh_t)
and the W1 projection run on the PE with the contraction over hidden:

    yT[gh, t]  = sum_hc W1c^T @ lhT_c          (6 matmuls, free=512)
    lg[0:1, t] = sum_hc wr_c^T @ lhT_c         (6 matmuls, free=512)

sigmoid(lg) -> gate row [1, 512]; a 1-row matmul against a ones vector
broadcasts it to [128, 512]; DVE multiplies it into yT while casting to
bf16 (ygT); ONE DMA-XBAR transpose per graph turns ygT into token-major
y chunks; pooling then contracts tokens directly: t1 = P'^T (g*y) =
(pool(gated lh)) @ W1 — no nf materialization, no PE transposes.  GCN
layers use \hat A as stationary; FC head is batched over all 8 graphs
with no transposes.  Phase A (projection matmuls) streams back-to-back
paced by the 8 graph DMAs (SP + GPSIMD SWDGE rings alternate), keeping
the PE continuously busy; phase B runs pooling + GCN in per-stage rounds
across graphs so every PE op's cross-engine inputs are a full round old.
"""

import os
import sys

import numpy as np

for _p in ("/opt/trn_rl_repo", "/root/.axon_site/_ro/trn_rl_repo"):
    if os.path.isdir(_p) and _p not in sys.path:
        sys.path.insert(0, _p)

import ml_dtypes  # noqa: E402
import concourse.bass as bass  # noqa: E402
import concourse.mybir as mybir  # noqa: E402
from concourse import tile  # noqa: E402
from concourse.bass_utils import run_bass_kernel_spmd  # noqa: E402

# Problem shapes (hardcoded per contest rules).
B, S, H = 64, 512, 768
N, E = 128, 1024
GH, FH, L = 128, 256, 2
NCORES = 8
BL = B // NCORES  # graphs per core
SC = S // 128     # subtoken chunks per graph
HC = H // 128     # BERT-hidden chunks
FC = (H + GH) // 128  # concat-feature chunks for the FC head

# mega-tensor column offsets (bf16)
MEG_LHT = 0             # [HC*S] = 3072: lhT[p, hc*S + t] = lh[t, hc*128+p]
MEG_PG = HC * S         # [SC*N] = 512: P' (one-hot * invc * dinv), token-major
MEG_AH = MEG_PG + SC * N  # [N] = 128: \hat A row block
MEG_L8 = MEG_AH + N     # [HC*S/2] = 1536: fp8 copy of lhT (2 vals per bf16 col)
MEG_W = MEG_L8 + HC * S // 2  # 5248 total

# consts column offsets (bf16)
C_W1 = 0                      # [HC*GH] = 768: [p, hc*128+j] = W1[hc*128+p, j]
C_W2 = C_W1 + HC * GH         # [GH]
C_WF1 = C_W2 + GH             # [FC*2*128] = 1792
C_WF2 = C_WF1 + FC * 2 * 128  # [2*L] = 4
C_CLS = C_WF2 + 2 * L         # [HC*BL] = 48
C_MEAN = C_CLS + HC * BL      # [1]
C_WR8 = C_MEAN + 12           # [HC*128/2] = 384: fp8 wr*64 bcast, DoubleRow
C_IDENT = C_WR8 + HC * 64     # [128]
C_W = C_IDENT + 128
WSCALE = 64.0                 # fp8 weight pre-scale (undone in sigmoid)

f32 = mybir.dt.float32
bf16 = mybir.dt.bfloat16
fp8 = mybir.dt.float8e4
AFT = mybir.ActivationFunctionType
ALU = mybir.AluOpType
BF16 = ml_dtypes.bfloat16

_CACHE = {}


def _split_multi_waits(nc: bass.Bass) -> int:
    """Walrus in this container accepts one sync-wait per instruction; split
    extra waits into single-wait EventSemaphore nops just before it."""
    n_split = 0
    for fn in nc.m.functions:
        for blk in fn.blocks:
            new_instrs = []
            changed = False
            for inst in blk.instructions:
                si = getattr(inst, "sync_info", None)
                if si is not None and si.on_wait is not None and len(si.on_wait) > 1:
                    waits = list(si.on_wait)
                    for j, w in enumerate(waits[:-1]):
                        ev = mybir.InstEventSemaphore(
                            name=f"{inst.name}_ws{j}",
                            ins=[], outs=[],
                            engine=inst.engine,
                            sync_info=mybir.SyncInfo(on_wait=[w], on_update=[]),
                        )
                        new_instrs.append(ev)
                    inst.sync_info = mybir.SyncInfo(
                        on_wait=[waits[-1]], on_update=list(si.on_update))
                    n_split += 1
                    changed = True
                new_instrs.append(inst)
            if changed:
                blk.instructions = new_instrs
    return n_split


def build_program(br_val: float, b1_zero: bool, b2_zero: bool,
                  bf1_zero: bool, bf2_zero: bool) -> bass.Bass:
    nc = bass.Bass()

    meg_d = nc.declare_dram_parameter("meg", [BL, 128, MEG_W], bf16, isOutput=False)
    consts_d = nc.declare_dram_parameter("consts", [128, C_W], bf16, isOutput=False)
    b1b_d = nc.declare_dram_parameter("b1b", [128, GH], f32, isOutput=False)
    b2b_d = nc.declare_dram_parameter("b2b", [128, GH], f32, isOutput=False)
    bf1b_d = nc.declare_dram_parameter("bf1b", [128, 2], f32, isOutput=False)
    bf2b_d = nc.declare_dram_parameter("bf2b", [L, 1], f32, isOutput=False)
    out_d = nc.declare_dram_parameter("out", [L, BL], f32, isOutput=True)

    with tile.TileContext(nc) as tc:
        with (
            tc.tile_pool(name="const", bufs=1) as cpool,
            tc.tile_pool(name="megp", bufs=BL) as megpool,
            tc.tile_pool(name="work", bufs=3) as wpool,
            tc.tile_pool(name="psY", bufs=2, space="PSUM") as psY,
            tc.tile_pool(name="psL", bufs=2, space="PSUM") as psL,
            tc.tile_pool(name="psB", bufs=4, space="PSUM") as psB,
        ):
            ctile = cpool.tile([128, C_W], bf16)
            nc.scalar.dma_start(ctile[:], consts_d[:])
            b1t = b2t = bf1t = bf2t = None
            if not b1_zero:
                b1t = cpool.tile([128, GH], f32, name="b1t")
                nc.scalar.dma_start(b1t[:], b1b_d[:])
            if not b2_zero:
                b2t = cpool.tile([128, GH], f32, name="b2t")
                nc.scalar.dma_start(b2t[:], b2b_d[:])
            if not bf1_zero:
                bf1t = cpool.tile([128, 2], f32, name="bf1t")
                nc.scalar.dma_start(bf1t[:], bf1b_d[:])
            if not bf2_zero:
                bf2t = cpool.tile([L, 1], f32, name="bf2t")
                nc.scalar.dma_start(bf2t[:], bf2b_d[:])
            catT6 = cpool.tile([128, BL], bf16)
            h1r = cpool.tile([128, 2, BL], bf16)

            # meg delivery: singles early (latency), pairs late (fewer DGE
            # gaps); sync HWDGE ring and gpsimd SWDGE ring alternate.
            megs = [None] * BL
            m0 = megpool.tile([128, MEG_W], bf16, tag="m0", bufs=1, name="m0")
            nc.sync.dma_start(m0[:], meg_d[0])
            megs[0] = m0
            p13 = megpool.tile([128, 2, MEG_W], bf16, tag="p13", bufs=1,
                               name="p13")
            nc.gpsimd.dma_start(p13[:], meg_d[1:4:2].rearrange("g p w -> p g w"))
            megs[1], megs[3] = p13[:, 0, :], p13[:, 1, :]
            m2 = megpool.tile([128, MEG_W], bf16, tag="m2", bufs=1, name="m2")
            nc.sync.dma_start(m2[:], meg_d[2])
            megs[2] = m2
            p46 = megpool.tile([128, 2, MEG_W], bf16, tag="p46", bufs=1,
                               name="p46")
            nc.sync.dma_start(p46[:], meg_d[4:7:2].rearrange("g p w -> p g w"))
            megs[4], megs[6] = p46[:, 0, :], p46[:, 1, :]
            p57 = megpool.tile([128, 2, MEG_W], bf16, tag="p57", bufs=1,
                               name="p57")
            nc.gpsimd.dma_start(p57[:], meg_d[5:8:2].rearrange("g p w -> p g w"))
            megs[5], megs[7] = p57[:, 0, :], p57[:, 1, :]

            W2c = ctile[:, C_W2:C_W2 + GH]
            MEAN = ctile[:, C_MEAN:C_MEAN + 1]
            IDENT = ctile[:, C_IDENT:C_IDENT + 128]

            yT_ps = [None] * BL
            lg_ps = [None] * BL
            gate_sb = [None] * BL
            y_sb = [None] * BL
            t1sb = [None] * BL
            x1 = [None] * BL
            x1t = [None] * BL
            t2sb = [None] * BL
            x2 = [None] * BL

            def relu_to(out_sb, z_ps, bias_tile, tag):
                if bias_tile is None:
                    nc.vector.tensor_scalar_max(out_sb[:], z_ps[:], 0.0)
                else:
                    tmp = wpool.tile([128, GH], f32, tag=tag + "b", bufs=2,
                                     name=tag + "b")
                    nc.vector.tensor_tensor(tmp[:], z_ps[:], bias_tile[:],
                                            ALU.add)
                    nc.vector.tensor_scalar_max(out_sb[:], tmp[:], 0.0)

            ygsb2 = [None] * (BL // 2)
            y2 = [None] * (BL // 2)

            def gate_into_y(g):
                """sigmoid on broadcast logits -> gate into yT; XBAR per pair."""
                k, half = g // 2, g % 2
                gb_sb = wpool.tile([128, S], bf16, tag="gbsb", bufs=2,
                                   name="gb_sb")
                nc.scalar.activation(gb_sb[:], lg_ps[g][:], AFT.Sigmoid,
                                     bias=float(br_val), scale=1.0 / WSCALE)
                if half == 0:
                    ygsb2[k] = wpool.tile([128, 2, S], bf16, tag="ygsb",
                                          bufs=2, name="ygsb")
                nc.vector.tensor_tensor(ygsb2[k][:, half, :], yT_ps[g][:],
                                        gb_sb[:], ALU.mult)
                if half == 1:
                    y2[k] = wpool.tile([128, 2 * SC, 128], bf16, tag="ysb",
                                       bufs=BL // 2, name="y_sb")
                    nc.scalar.dma_start(y2[k][:], ygsb2[k][:], transpose=True)
                    y_sb[2 * k] = y2[k][:, 0:SC, :]
                    y_sb[2 * k + 1] = y2[k][:, SC:2 * SC, :]

            def pool_g(g):
                t1_ps = psB.tile([128, GH], f32, tag="mm", name="t1_ps")
                for c in range(SC):
                    nc.tensor.matmul(
                        t1_ps[:],
                        megs[g][:, MEG_PG + c * N:MEG_PG + (c + 1) * N],
                        y_sb[g][:, c, :], start=(c == 0), stop=(c == SC - 1))
                t1sb[g] = wpool.tile([128, GH], bf16, tag="t1sb", bufs=BL,
                                     name="t1sb")
                nc.scalar.copy(t1sb[g][:], t1_ps[:])

            # ---- phase A: projection + gate + pooling, DMA-paced ----
            for s in range(BL):
                yT_ps[s] = psY.tile([128, S], f32, tag="yt", name="yT_ps")
                lg_ps[s] = psL.tile([128, S], f32, tag="lg", name="lg_ps")
                for hc in range(HC):
                    lht_c = megs[s][:, MEG_LHT + hc * S:MEG_LHT + (hc + 1) * S]
                    nc.tensor.matmul(
                        yT_ps[s][:],
                        ctile[:, C_W1 + hc * GH:C_W1 + (hc + 1) * GH],
                        lht_c, start=(hc == 0), stop=(hc == HC - 1))
                for j in range(HC // 2):
                    nc.tensor.matmul(
                        lg_ps[s][:],
                        ctile[:, C_WR8 + j * 128:C_WR8 + (j + 1) * 128]
                        .bitcast(fp8).rearrange("p (i m) -> p i m", i=2),
                        megs[s][:, MEG_L8 + j * S:MEG_L8 + (j + 1) * S]
                        .bitcast(fp8).rearrange("p (i t) -> p i t", i=2),
                        start=(j == 0), stop=(j == HC // 2 - 1),
                        perf_mode=mybir.MatmulPerfMode.DoubleRow)
                if s >= 1:
                    gate_into_y(s - 1)
                if s >= 3 and s % 2 == 1:
                    k = (s - 3) // 2
                    pool_g(2 * k)
                    pool_g(2 * k + 1)
            gate_into_y(BL - 1)
            pool_g(BL - 2)
            pool_g(BL - 1)

            # ---- phase B: GCN in rounds across graphs ----
            for g in range(BL):
                z_ps = psB.tile([128, GH], f32, tag="mm", name="z_ps")
                nc.tensor.matmul(z_ps[:], megs[g][:, MEG_AH:MEG_AH + N],
                                 t1sb[g][:], start=True, stop=True)
                x1[g] = wpool.tile([128, GH], bf16, tag="x1", bufs=BL,
                                   name="x1")
                relu_to(x1[g], z_ps, b1t, "x1")
            for g in range(BL):
                xt_ps = psB.tile([128, GH], bf16, tag="mm", name="xt_ps")
                nc.tensor.transpose(xt_ps[:], x1[g][:], IDENT)
                x1t[g] = wpool.tile([128, GH], bf16, tag="x1t", bufs=BL,
                                    name="x1t")
                nc.vector.tensor_copy(x1t[g][:], xt_ps[:])
            for g in range(BL):
                t2_ps = psB.tile([128, GH], f32, tag="mm", name="t2_ps")
                nc.tensor.matmul(t2_ps[:], x1t[g][:], W2c,
                                 start=True, stop=True)
                t2sb[g] = wpool.tile([128, GH], bf16, tag="t2sb", bufs=BL,
                                     name="t2sb")
                nc.scalar.copy(t2sb[g][:], t2_ps[:])
            for g in range(BL):
                z2_ps = psB.tile([128, GH], f32, tag="mm", name="z2_ps")
                nc.tensor.matmul(z2_ps[:], megs[g][:, MEG_AH:MEG_AH + N],
                                 t2sb[g][:], start=True, stop=True)
                x2[g] = wpool.tile([128, GH], bf16, tag="x2", bufs=BL,
                                   name="x2")
                relu_to(x2[g], z2_ps, b2t, "x2")
            for g in range(BL):
                mp_ps = psB.tile([128, 1], f32, tag="mm", name="mp_ps")
                nc.tensor.matmul(mp_ps[:], x2[g][:], MEAN,
                                 start=True, stop=True)
                nc.vector.tensor_copy(catT6[:, g:g + 1], mp_ps[:])

            # ---------- FC head over all BL graphs ----------
            h1_ps = []
            for hh in range(2):
                hp = psB.tile([128, BL], f32, tag="mm", name=f"h1_ps{hh}")
                for c in range(FC):
                    lhsT = ctile[:, C_WF1 + (c * 2 + hh) * 128:
                                 C_WF1 + (c * 2 + hh + 1) * 128]
                    rhs = (ctile[:, C_CLS + c * BL:C_CLS + (c + 1) * BL]
                           if c < HC else catT6[:])
                    nc.tensor.matmul(hp[:], lhsT, rhs, start=(c == 0),
                                     stop=(c == FC - 1))
                h1_ps.append(hp)
            for hh in range(2):
                if bf1t is None:
                    nc.vector.tensor_scalar_max(h1r[:, hh, :], h1_ps[hh][:],
                                                0.0)
                else:
                    nc.vector.tensor_scalar(h1r[:, hh, :], h1_ps[hh][:],
                                            bf1t[:, hh:hh + 1], 0.0,
                                            ALU.add, ALU.max)
            out_ps = psB.tile([L, BL], f32, tag="mm", name="out_ps")
            for hh in range(2):
                nc.tensor.matmul(out_ps[:],
                                 ctile[:, C_WF2 + hh * L:C_WF2 + (hh + 1) * L],
                                 h1r[:, hh, :], start=(hh == 0),
                                 stop=(hh == 1))
            outs = cpool.tile([L, BL], f32)
            if bf2t is None:
                nc.vector.tensor_copy(outs[:], out_ps[:])
            else:
                nc.vector.tensor_scalar_add(outs[:], out_ps[:], bf2t[:])
            nc.sync.dma_start(out_d[:], outs[:])

    _split_multi_waits(nc)
    return nc


def _prepare_in_maps(inputs):
    lh = np.ascontiguousarray(np.asarray(inputs["last_hidden"], dtype=np.float32))
    submap = np.asarray(inputs["submap"]).astype(np.int64)
    edge_index = np.asarray(inputs["edge_index"]).astype(np.int64)
    assert lh.shape == (B, S, H)
    assert int(inputs.get("num_nodes", N)) == N

    wr = np.asarray(inputs["wr"], dtype=np.float32)
    br = float(np.asarray(inputs["br"], dtype=np.float32))
    W1 = np.asarray(inputs["W1"], dtype=np.float32)
    b1 = np.asarray(inputs["b1"], dtype=np.float32)
    W2 = np.asarray(inputs["W2"], dtype=np.float32)
    b2 = np.asarray(inputs["b2"], dtype=np.float32)
    Wf1 = np.asarray(inputs["Wf1"], dtype=np.float32)
    bf1 = np.asarray(inputs["bf1"], dtype=np.float32)
    Wf2 = np.asarray(inputs["Wf2"], dtype=np.float32)
    bf2 = np.asarray(inputs["bf2"], dtype=np.float32)

    # ---- host-side index prep: adjacency, degrees, counts ----
    src = edge_index[:, 0, :]
    dst = edge_index[:, 1, :]
    flat = (np.arange(B, dtype=np.int64)[:, None] * (N * N) + src * N + dst)
    A = np.bincount(flat.reshape(-1), minlength=B * N * N).astype(np.float32)
    A = A.reshape(B, N, N) + np.eye(N, dtype=np.float32)[None]
    deg = A.sum(axis=1)                      # in-degree incl self-loops
    dinv = 1.0 / np.sqrt(deg)
    ahat = A * dinv[:, :, None] * dinv[:, None, :]

    cflat = np.arange(B, dtype=np.int64)[:, None] * N + submap
    cnt = np.bincount(cflat.reshape(-1), minlength=B * N).astype(np.float32)
    invc = 1.0 / np.maximum(cnt.reshape(B, N), 1.0)

    P = (submap[:, :, None] == np.arange(N)[None, None, :]).astype(np.float32)
    P *= (invc * dinv)[:, None, :]

    # ---- mega-tensor assembly (bf16 + packed fp8) ----
    FP8 = np.dtype(mybir.dt.np(fp8))
    lht = lh.astype(BF16).reshape(B, S, HC, 128).transpose(0, 3, 2, 1)
    p_r = P.astype(BF16).reshape(B, SC, 128, N).transpose(0, 2, 1, 3)
    lht8 = np.ascontiguousarray(
        lh.astype(FP8).reshape(B, S, HC, 128).transpose(0, 3, 2, 1))
    meg = np.empty((B, 128, MEG_W), dtype=BF16)
    meg[:, :, MEG_LHT:MEG_PG] = lht.reshape(B, 128, HC * S)
    meg[:, :, MEG_PG:MEG_AH] = p_r.reshape(B, 128, SC * N)
    meg[:, :, MEG_AH:MEG_L8] = ahat.astype(BF16)
    meg[:, :, MEG_L8:MEG_W] = lht8.reshape(B, 128, HC * S).view(BF16)

    # ---- consts (bf16), cls block differs per core ----
    consts = np.zeros((128, C_W), dtype=np.float32)
    consts[:, C_W1:C_W1 + HC * GH] = (
        W1.reshape(HC, 128, GH).transpose(1, 0, 2).reshape(128, HC * GH))
    consts[:, C_W2:C_W2 + GH] = W2
    consts[:, C_WF1:C_WF1 + FC * 2 * 128] = (
        Wf1.reshape(FC, 128, 2, 128).transpose(1, 0, 2, 3).reshape(128, -1))
    consts[:, C_WF2:C_WF2 + 2 * L] = (
        Wf2.reshape(2, 128, L).transpose(1, 0, 2).reshape(128, 2 * L))
    consts[:, C_MEAN] = 1.0 / N
    consts[:, C_IDENT:C_IDENT + 128] = np.eye(128, dtype=np.float32)

    wr8 = np.ascontiguousarray(np.broadcast_to(
        (wr * WSCALE).astype(FP8).reshape(HC, 128).T[:, :, None],
        (128, HC, 128)))
    wr8_packed = wr8.reshape(128, HC * 128).view(BF16)

    b1b = np.ascontiguousarray(np.broadcast_to(b1, (128, GH)).astype(np.float32))
    b2b = np.ascontiguousarray(np.broadcast_to(b2, (128, GH)).astype(np.float32))
    bf1b = np.ascontiguousarray(bf1.reshape(2, 128).T.astype(np.float32))
    bf2b = np.ascontiguousarray(bf2.reshape(L, 1).astype(np.float32))

    in_maps = []
    for i in range(NCORES):
        sl = slice(i * BL, (i + 1) * BL)
        ci = consts.copy()
        ci[:, C_CLS:C_CLS + HC * BL] = (
            lh[sl, 0, :].reshape(BL, HC, 128).transpose(2, 1, 0)
            .reshape(128, HC * BL))
        cb = ci.astype(BF16)
        cb[:, C_WR8:C_WR8 + HC * 64] = wr8_packed
        in_maps.append({
            "meg": np.ascontiguousarray(meg[sl]),
            "consts": cb,
            "b1b": b1b, "b2b": b2b, "bf1b": bf1b, "bf2b": bf2b,
        })
    flags = (br, bool(np.all(b1 == 0)), bool(np.all(b2 == 0)),
             bool(np.all(bf1 == 0)), bool(np.all(bf2 == 0)))
    return in_maps, flags


def _run(inputs, trace=False):
    in_maps, flags = _prepare_in_maps(inputs)
    key = ("prog",) + flags
    if key not in _CACHE:
        _CACHE[key] = build_program(*flags)
    nc = _CACHE[key]
    res = run_bass_kernel_spmd(nc, in_maps, list(range(NCORES)), trace=trace)
    out = np.concatenate(
        [np.asarray(res.results[i]["out"]).T for i in range(NCORES)],
        axis=0).astype(np.float32)
    return out, res


def kernel(**inputs) -> np.ndarray:
    out, _ = _run(inputs, trace=False)
    return out
